# revision 3
# baseline (speedup 1.0000x reference)
"""Trainium2 Bass kernel v2 for nn_BaselineGRU: 2-layer GRU (B=16,T=64,NN=4096,
H=1024) + decoder on 8 NeuronCores.

Strategy: gate-dimension sharding (each core owns a 128-wide hidden slice),
TRANSPOSED gate layout ([hidden-on-partitions, batch-free]) so DVE runs at
full width and the per-step GEMMs are stationary-weight (bf16 for fast FWL
weight loads) with tiny N=16 moving operands.  Both layers' hidden-state
chunks are exchanged with ONE fused AllGather per timestep ([256,16] bf16,
Shared-address output buffers for the fast collective path — 66 collectives
total vs 129 in the previous kernel).  Layer-1 runs 2 slots behind layer-0
with its input projection (gx1) prefetched one slot early, so each chain's
serial loop is [own-GEMM -> gates -> cast -> AllGather] only.  All weights
and h are bf16 (rel err ~5e-3), gates math in fp32.
"""
import numpy as np

import concourse.bacc as bacc
import concourse.tile as tile
import concourse.mybir as mybir
from concourse import bass_utils

B, T, NN, H = 16, 64, 4096, 1024
NCR = 8
HC = H // NCR        # 128 hidden per core
KH = H // 128        # 8 K-chunks over hidden
KX = NN // 128       # 32 K-chunks over input features
DC = NN // NCR       # 512 decoder rows per core
TB = T * B           # 1024 tokens
fp32 = mybir.dt.float32
bf16 = mybir.dt.bfloat16

_CACHE = {}


def _preseed_sched_sim(nc, sem_handles, value=1 << 20):
    """Scheduling-pass CoreSim can't model remote sem increments; pre-seed
    those sems huge so its waits pass instantly. HW waits are unaffected."""
    import concourse.bass_interp as bi
    ids = [(h.num, h.name) for h in sem_handles]
    orig = bi.CoreSim.simulate

    def patched(self, *a, **kw):
        if getattr(self, "scheduling_pass", False) and self.module is nc:
            for num, name in ids:
                self.update_semaphore(mybir.SyncUpdate(
                    sync_type="semaphore", id=num, ant_name=name,
                    update_mode="sem-add-imm", update_value=value))
        return orig(self, *a, **kw)

    bi.CoreSim.simulate = patched
    return orig


def _build(comm="cc", TT=T):
    nc = bacc.Bacc("TRN2", target_bir_lowering=False, debug=False,
                   enable_asserts=False, num_devices=NCR)
    S = mybir.ActivationFunctionType.Sigmoid
    TA = mybir.ActivationFunctionType.Tanh
    d = {}
    d["xd"] = nc.dram_tensor("xd", [KX, 128, TB], bf16, kind="ExternalInput").ap()
    d["wih0T"] = nc.dram_tensor("wih0T", [128, 3 * KX * 128], bf16,
                                kind="ExternalInput").ap()
    d["whh0T"] = nc.dram_tensor("whh0T", [128, 3 * KH * 128], bf16,
                                kind="ExternalInput").ap()
    d["wih1T"] = nc.dram_tensor("wih1T", [128, 3 * KH * 128], bf16,
                                kind="ExternalInput").ap()
    d["whh1T"] = nc.dram_tensor("whh1T", [128, 3 * KH * 128], bf16,
                                kind="ExternalInput").ap()
    d["decw"] = nc.dram_tensor("decw", [128, KH * DC], bf16,
                               kind="ExternalInput").ap()
    for nm in ("bih0", "b0h", "b1x", "b1h"):
        d[nm] = nc.dram_tensor(nm, [1, 3 * 128], bf16, kind="ExternalInput").ap()
    d["decb"] = nc.dram_tensor("decb", [1, DC], bf16, kind="ExternalInput").ap()
    d["ones"] = nc.dram_tensor("ones", [1, 512], bf16, kind="ExternalInput").ap()
    out_d = nc.dram_tensor("out", [B, DC], fp32, kind="ExternalOutput").ap()

    with tile.TileContext(nc) as tc:
        with tc.tile_pool(name="wsb", bufs=1) as wsb, \
             tc.tile_pool(name="gp", bufs=3) as gp, \
             tc.tile_pool(name="agd", bufs=4, space="DRAM") as agd:

            # ---- persistent SBUF ----
            wih0T = wsb.tile([128, 3 * KX * 128], bf16, tag="wih0T")
            whh0T = wsb.tile([128, 3 * KH * 128], bf16, tag="whh0T")
            wih1T = wsb.tile([128, 3 * KH * 128], bf16, tag="wih1T")
            whh1T = wsb.tile([128, 3 * KH * 128], bf16, tag="whh1T")
            decw = wsb.tile([128, KH * DC], bf16, tag="decw")
            for t_, nm in ((wih0T, "wih0T"), (whh0T, "whh0T"), (wih1T, "wih1T"),
                           (whh1T, "whh1T"), (decw, "decw")):
                nc.sync.dma_start(out=t_[:], in_=d[nm])
            bias = {}
            for nm in ("bih0", "b0h", "b1x", "b1h"):
                bias[nm] = wsb.tile([1, 3 * 128], bf16, tag=nm, name=nm)
                nc.sync.dma_start(out=bias[nm][:], in_=d[nm])
            decb = wsb.tile([1, DC], bf16, tag="decb")
            nc.sync.dma_start(out=decb[:], in_=d["decb"])
            ones = wsb.tile([1, 512], bf16, tag="ones")
            nc.sync.dma_start(out=ones[:], in_=d["ones"])

            gx0g = [wsb.tile([128, 16 * T], fp32, tag=f"gx0g{g}",
                             name=f"gx0g{g}") for g in range(3)]
            gxs = [wsb.tile([128, 48], fp32, tag=f"gxs{p}", name=f"gxs{p}")
                   for p in range(2)]
            # fused comm tiles: recvf cols = u*128 + r*16 + b (u: 0=h1,1=h2);
            # sendf cols = u*16 + b
            recvf = [wsb.tile([128, 256], bf16, tag=f"recvf{p}",
                              name=f"recvf{p}") for p in range(3)]
            sendf = [wsb.tile([128, 32], bf16, tag=f"sendf{p}",
                              name=f"sendf{p}") for p in range(2)]
            zero = wsb.tile([128, 16], fp32, tag="zero")
            nc.vector.memset(zero[:], 0.0)
            for t_ in recvf + sendf:
                nc.vector.memset(t_[:], 0.0)

            agoutS = [nc.dram_tensor(f"agoutS{j}", [256 * NCR, 16], bf16,
                                     kind="Internal", addr_space="Shared")
                      for j in range(4)]

            def emit_fused_ag(s):
                """One AG per slot: [h1(s); h2(s-2)] chunks [256,16]."""
                agin = agd.tile([256, 16], bf16, tag="agin")
                nc.sync.dma_start(
                    out=agin[:].rearrange("(u p) b -> p u b", u=2),
                    in_=sendf[s % 2][:].rearrange("p (u b) -> p u b", u=2))
                ago = agoutS[s % 4].ap()
                nc.gpsimd.collective_compute(
                    "AllGather", mybir.AluOpType.bypass,
                    replica_groups=[list(range(NCR))],
                    ins=[agin.opt()], outs=[ago])
                # agout rows: rank r at [256r:256r+256]; h1 first 128.
                a4 = ago.rearrange("(r u p) b -> u p r b", u=2, p=128)
                nc.sync.dma_start(
                    out=recvf[s % 3][:, 0:128].rearrange("p (r b) -> p r b",
                                                         r=NCR),
                    in_=a4[0])
                nc.sync.dma_start(
                    out=recvf[s % 3][:, 128:256].rearrange("p (r b) -> p r b",
                                                           r=NCR),
                    in_=a4[1])

            # ---- layer-0 input projection: gx0g[g][:, 16s+b] ----
            with tc.tile_pool(name="ppj", bufs=1, space="PSUM") as ppj, \
                 tc.tile_pool(name="xp", bufs=2) as xp:
                pj = []
                for g in range(3):
                    for hf in range(2):
                        t_ = ppj.tile([128, 512], fp32, tag=f"pj{g}{hf}",
                                      name=f"pj{g}{hf}")
                        pj.append(t_)
                        nc.tensor.matmul(t_[:],
                                         bias["bih0"][:, 128 * g:128 * (g + 1)],
                                         ones[0:1, 0:512], start=True, stop=False)
                for k in range(KX):
                    xt = xp.tile([128, TB], bf16, tag="xt")
                    nc.sync.dma_start(out=xt[:], in_=d["xd"][k])
                    for g in range(3):
                        for hf in range(2):
                            nc.tensor.matmul(
                                pj[2 * g + hf][:],
                                wih0T[:, (g * KX + k) * 128:(g * KX + k + 1) * 128],
                                xt[:, 512 * hf:512 * (hf + 1)],
                                start=False, stop=(k == KX - 1))
                for g in range(3):
                    for hf in range(2):
                        nc.vector.tensor_copy(gx0g[g][:, 512 * hf:512 * (hf + 1)],
                                              pj[2 * g + hf][:])

            # PSUM pool for the scan opens after the projection pool closed
            # so its banks can be reused (stack allocator).
            from contextlib import ExitStack
            _pss = ExitStack()
            psp = _pss.enter_context(
                tc.tile_pool(name="psp", bufs=1, space="PSUM"))

            # ---- gates in transposed layout ----
            def gates_T(tag, gh_ps, gxr, gxz, gxn, h_old):
                pre = gp.tile([128, 32], fp32, tag=f"{tag}pre")
                nc.vector.tensor_add(pre[:, 0:16], gxr, gh_ps[:, 0:16])
                nc.vector.tensor_add(pre[:, 16:32], gxz, gh_ps[:, 16:32])
                rz = gp.tile([128, 32], fp32, tag=f"{tag}rz")
                nc.scalar.activation(rz[:], pre[:], S)
                c0 = gp.tile([128, 16], fp32, tag=f"{tag}c0")
                nc.vector.tensor_mul(c0[:], rz[:, 0:16], gh_ps[:, 32:48])
                d0 = gp.tile([128, 16], fp32, tag=f"{tag}d0")
                nc.vector.tensor_add(d0[:], gxn, c0[:])
                n0 = gp.tile([128, 16], fp32, tag=f"{tag}n0")
                nc.scalar.activation(n0[:], d0[:], TA)
                e0 = gp.tile([128, 16], fp32, tag=f"{tag}e0")
                nc.vector.tensor_sub(e0[:], h_old[:], n0[:])
                f0 = gp.tile([128, 16], fp32, tag=f"{tag}f0")
                nc.vector.tensor_mul(f0[:], rz[:, 16:32], e0[:])
                hn = gp.tile([128, 16], fp32, tag=f"{tag}hn")
                nc.vector.tensor_add(hn[:], n0[:], f0[:])
                return hn

            def gemm48(ps, wT, bsl, rvt, off, with_h):
                """ps[:,16g:+16] = bias_g + sum_k wT(g,k).T @ rvt[:,off+16k:+16]"""
                for g in range(3):
                    nc.tensor.matmul(ps[:, 16 * g:16 * (g + 1)],
                                     bsl[:, 128 * g:128 * (g + 1)],
                                     ones[0:1, 0:16],
                                     start=True, stop=not with_h)
                    if with_h:
                        for k in range(KH):
                            nc.tensor.matmul(
                                ps[:, 16 * g:16 * (g + 1)],
                                wT[:, (g * KH + k) * 128:(g * KH + k + 1) * 128],
                                rvt[:, off + 16 * k:off + 16 * (k + 1)],
                                start=False, stop=(k == KH - 1))

            # ---- scan: slots 0..T+2 ----
            V1, V2 = {}, {}
            v1 = v2 = 0
            h1_old = zero
            h2_old = zero
            T_ = TT
            for s in range(T_ + 3):
                # chain-2 recurrent GEMM: gh1 for h2(s-2)
                if 2 <= s <= T_ + 1:
                    psh = psp.tile([128, 48], fp32, tag=f"psh{s % 2}",
                                   name=f"psh{s % 2}")
                    gemm48(psh, whh1T, bias["b1h"], recvf[(s - 1) % 3], 128, s >= 3)
                # chain-1 recurrent GEMM: gh0 for h1(s)
                if s < T_:
                    ps0 = psp.tile([128, 48], fp32, tag=f"ps0{s % 2}",
                                   name=f"ps0{s % 2}")
                    gemm48(ps0, whh0T, bias["b0h"], recvf[(s - 1) % 3], 0, s >= 1)
                # chain-2 input GEMM prefetch: gx1 for h2(s-1), used next slot
                if 1 <= s <= T_:
                    psx = psp.tile([128, 48], fp32, tag=f"psx{s % 2}",
                                   name=f"psx{s % 2}")
                    gemm48(psx, wih1T, bias["b1x"], recvf[(s - 1) % 3], 0, True)

                # chain-2 gates + send h2(s-2)
                if 2 <= s <= T_ + 1:
                    gxc = gxs[s % 2]
                    h2n = gates_T("l1", psh, gxc[:, 0:16], gxc[:, 16:32],
                                  gxc[:, 32:48], h2_old)
                    h2_old = h2n
                    nc.vector.tensor_copy(sendf[s % 2][:, 16:32], h2n[:])
                    v2 += 16
                    V2[s] = v2
                # chain-1 gates + send h1(s)
                if s < T_:
                    h1n = gates_T("l0", ps0, gx0g[0][:, 16 * s:16 * (s + 1)],
                                  gx0g[1][:, 16 * s:16 * (s + 1)],
                                  gx0g[2][:, 16 * s:16 * (s + 1)], h1_old)
                    h1_old = h1n
                    nc.vector.tensor_copy(sendf[s % 2][:, 0:16], h1n[:])
                    v1 += 16
                    V1[s] = v1
                if s <= T_ + 1:
                    emit_fused_ag(s)
                # evict psx to SBUF for next slot's chain-2 gates
                if 1 <= s <= T_:
                    nc.vector.tensor_copy(gxs[(s + 1) % 2][:], psx[:])

            # ---- decoder: out = h2(T-1) @ dec_w_c.T + dec_b_c ----
            rvt = recvf[(T_ + 1) % 3]
            pd = psp.tile([16, DC], fp32, tag="dec")
            for k in range(KH):
                nc.tensor.matmul(pd[:], rvt[:, 128 + 16 * k:128 + 16 * (k + 1)],
                                 decw[:, k * DC:(k + 1) * DC],
                                 start=(k == 0), stop=False)
            nc.tensor.matmul(pd[:], ones[0:1, 0:16], decb[:],
                             start=False, stop=True)
            od = gp.tile([16, DC], fp32, tag="od")
            nc.vector.tensor_copy(od[:], pd[:])
            nc.sync.dma_start(out=out_d, in_=od[:])
            _pss.close()

    nc.compile()
    return nc


def _prep_in_maps(x, w_ih_l0, w_hh_l0, b_ih_l0, b_hh_l0,
                  w_ih_l1, w_hh_l1, b_ih_l1, b_hh_l1, dec_w, dec_b):
    bfnp = mybir.dt.np(bf16)
    x = np.asarray(x, np.float32)
    xt = np.ascontiguousarray(x.transpose(2, 1, 0).reshape(NN, TB))
    xd = np.ascontiguousarray(xt.reshape(KX, 128, TB)).astype(bfnp)

    def pack_T(w, c, kchunks):
        w = np.asarray(w, np.float32)
        out = np.empty((128, 3 * kchunks * 128), np.float32)
        for g in range(3):
            wg = w[g * H + c * HC: g * H + (c + 1) * HC, :]       # [128, K]
            blk = wg.reshape(128, kchunks, 128).transpose(2, 1, 0)  # [p,k,m]
            out[:, g * kchunks * 128:(g + 1) * kchunks * 128] = \
                blk.reshape(128, kchunks * 128)
        return out.astype(bfnp)

    def bias_slice(b, c):
        b = np.asarray(b, np.float32)
        return np.concatenate(
            [b[g * H + c * HC: g * H + (c + 1) * HC] for g in range(3)]
        )[None, :].astype(bfnp)

    def pack_kT(w_rows, kchunks, ncols):
        wT = np.ascontiguousarray(np.asarray(w_rows, np.float32).T)
        return np.ascontiguousarray(
            wT.reshape(kchunks, 128, ncols).transpose(1, 0, 2)
            .reshape(128, kchunks * ncols)).astype(bfnp)

    ones = np.ones((1, 512), np.float32).astype(bfnp)
    dec_w = np.asarray(dec_w, np.float32)
    dec_b = np.asarray(dec_b, np.float32)
    in_maps = []
    for c in range(NCR):
        drows = slice(c * DC, (c + 1) * DC)
        in_maps.append({
            "xd": xd,
            "wih0T": pack_T(w_ih_l0, c, KX),
            "whh0T": pack_T(w_hh_l0, c, KH),
            "wih1T": pack_T(w_ih_l1, c, KH),
            "whh1T": pack_T(w_hh_l1, c, KH),
            "decw": pack_kT(dec_w[drows], KH, DC),
            "bih0": bias_slice(b_ih_l0, c),
            "b0h": bias_slice(b_hh_l0, c),
            "b1x": bias_slice(b_ih_l1, c),
            "b1h": bias_slice(b_hh_l1, c),
            "decb": dec_b[drows][None, :].astype(bfnp),
            "ones": ones,
        })
    return in_maps


def kernel(**kw):
    import os
    comm = _CACHE.get("comm", os.environ.get("KV2_COMM", "cc"))
    key = f"nc_{comm}"
    if key not in _CACHE:
        _CACHE[key] = _build(comm)
    nc = _CACHE[key]
    _CACHE["nc"] = nc
    in_maps = _prep_in_maps(**kw)
    _CACHE["last_in_maps"] = in_maps
    res = bass_utils.run_bass_kernel_spmd(
        nc, in_maps, core_ids=list(range(NCR)), trace=False)
    out = np.concatenate([res.results[c]["out"] for c in range(NCR)], axis=1)
    return out
